# revision 13
# baseline (speedup 1.0000x reference)
"""Trainium2 Bass kernel for nn_InternalMAFE_59270548684863 (v2b).

Output depends only on branch 1 (p=7, n=288): out = o1 @ proj_len_w.T + plb.
Batch-sharded over 8 cores (512 rows each); softmax over batch needs one
AllReduce of per-(j,i) exp-sums.

v2b strategy vs baseline:
  - proj_len_w work is PAIR-SPLIT: cores 2k/2k+1 each transpose half of
    plw^T (their 512 seq cols) and exchange via pair-shared DRAM, riding
    the exp-sum AllReduce as the sync barrier.  Halves plw HBM traffic,
    de-interleave cost and PE transpose count per core.
  - proj_len_b folded into the projection as a rank-1 K-tile (ones x plb).
  - Weight-stationary loop orders (lhsT constant over step groups) so
    LDWEIGHTS stays hidden; transposes on PE stay dense (warm HAM).
  - E tiles reused in-place for ys (numerator) and the scan state.
All transposes on the PE (dma_start_transpose measured 1.2us/op - too slow).
"""

import math

import numpy as np

import concourse.bacc as bacc
import concourse.bass as bass
import concourse.masks as masks
import concourse.mybir as mybir
import concourse.tile as tile
from concourse.bass_utils import run_bass_kernel_spmd
from concourse.tile import add_dep_helper

N_CORES = 8
B = 4096
BL = B // N_CORES  # 512 rows per core
INP = 2016
P1 = 7
N1 = 288
SEQ = 1024
SCALE = 1.0 / math.sqrt(N1)
SHIFT = -50.0
F32 = mybir.dt.float32
BF16 = mybir.dt.bfloat16
CH = [(0, 128), (128, 128), (256, 32)]
AF = mybir.ActivationFunctionType
SG = [(0, 4), (4, 3)]  # step groups (start, count) for PSUM pressure


def build():
    nc = bacc.Bacc(
        "TRN2", target_bir_lowering=False, debug=False, num_devices=N_CORES
    )
    x = nc.dram_tensor("x", [BL, INP], F32, kind="ExternalInput").ap()
    wk = nc.dram_tensor("w_k1", [N1, N1], F32, kind="ExternalInput").ap()
    wv = nc.dram_tensor("w_v1", [N1, N1], F32, kind="ExternalInput").ap()
    h1 = nc.dram_tensor("h1", [N1, N1], F32, kind="ExternalInput").ap()
    a1 = nc.dram_tensor("alpha1", [1], F32, kind="ExternalInput").ap()
    a2 = nc.dram_tensor("alpha2", [1], F32, kind="ExternalInput").ap()
    b1 = nc.dram_tensor("beta1", [1], F32, kind="ExternalInput").ap()
    b2 = nc.dram_tensor("beta2", [1], F32, kind="ExternalInput").ap()
    plw = nc.dram_tensor("proj_len_w", [SEQ, INP], F32, kind="ExternalInput").ap()
    plb = nc.dram_tensor("proj_len_b", [SEQ], F32, kind="ExternalInput").ap()
    out = nc.dram_tensor("out", [BL, SEQ], F32, kind="ExternalOutput").ap()

    with tile.TileContext(nc) as tc:
        with (
            tc.tile_pool(name="const", bufs=1) as cpool,
            tc.tile_pool(name="xiT", bufs=1) as xtpool,
            tc.tile_pool(name="ee", bufs=1) as epool,
            tc.tile_pool(name="rk", bufs=1) as rkpool,
            tc.tile_pool(name="dram", bufs=1, space="DRAM") as dpool,
        ):
            # ---------------- scalars / constants ----------------
            pid = nc.sync.partition_id()
            par = nc.sync.snap(pid % 2, min_val=0, max_val=1)
            other = nc.sync.snap((pid + 1) % 2, min_val=0, max_val=1)

            scal = cpool.tile([1, 4], F32, tag="scal", name="scal")
            for idx, ap in enumerate((a1, a2, b1, b2)):
                nc.sync.dma_start(scal[0:1, idx : idx + 1], ap[:])
            onesf = cpool.tile([1, 128], F32, tag="onesf", name="onesf")
            nc.vector.memset(onesf[:], 1.0)
            ident = cpool.tile([128, 128], BF16, tag="ident", name="ident")
            masks.make_identity(nc, ident[:])

            densb = cpool.tile([128, 24], F32, tag="densb", name="densb")
            nc.vector.memset(densb[:], 0.0)
            shiftc = cpool.tile([128, 1], F32, tag="shiftc", name="shiftc")
            nc.vector.memset(shiftc[:], SHIFT)
            den_all = cpool.tile([128, 24], F32, tag="den_all", name="den_all")
            recip = cpool.tile([128, 24], F32, tag="recip", name="recip")

            cc_in = dpool.tile([128, 24], F32, name="cc_in")
            cc_out = dpool.tile([128, 24], F32, addr_space="Shared", name="cc_out")
            # pair-shared buffers for plw^T halves (21 ragged + 1 bias row)
            plws = [
                dpool.tile([2, cnt, 512], BF16, addr_space="Shared", name=f"plws{i}_{c}")
                for i in range(P1)
                for c, (n0, cnt) in enumerate(CH)
            ]
            plwsb = dpool.tile([2, 1, 512], BF16, addr_space="Shared", name="plwsb")

            with (
                tc.tile_pool(name="wst", bufs=3) as wstpool,
                tc.tile_pool(name="xn", bufs=2) as xpool,
                tc.tile_pool(name="xnb", bufs=4) as xbpool,
                tc.tile_pool(name="plwf", bufs=2) as plwfpool,
                tc.tile_pool(name="plwb", bufs=2) as plwbpool,
                tc.tile_pool(name="psA", bufs=6, space="PSUM") as psA,
                tc.tile_pool(name="psT", bufs=2, space="PSUM") as psT,
            ):
                # gate scalars broadcast to 128 partitions via PE
                pbc = psA.tile([128, 512], F32, tag="ps", name="ps_bc")
                nc.tensor.matmul(pbc[:, 0:4], onesf[:], scal[:], start=True, stop=True)
                bcast = cpool.tile([128, 4], F32, tag="bcast", name="bcast")
                nc.vector.tensor_copy(bcast[:], pbc[:, 0:4])

                # ---------------- weights: load, bf16, PE transpose --------
                wkb, wvb, h1b = [], [], []
                for t, (m0, mc) in enumerate(CH):
                    for src, dstlist, nm in ((wk, wkb, "wkb"), (wv, wvb, "wvb"), (h1, h1b, "h1b")):
                        wtf = wstpool.tile([mc, N1], F32, tag="wtmp", name="wtmp")
                        nc.sync.dma_start(wtf[:], src[m0 : m0 + mc, :])
                        wt = cpool.tile([mc, N1], BF16, tag=f"{nm}{t}", name=f"{nm}{t}")
                        nc.vector.tensor_copy(wt[:], wtf[:])
                        dstlist.append(wt)

                # wkT[lt][l, m] = wk[m, l], h1T[lt][l, j] = h1[j, l]
                wkT, h1T = [], []
                for lt, (l0, lc) in enumerate(CH):
                    psk = psT.tile([128, 512], BF16, tag="tp", name="tp")
                    psh = psT.tile([128, 512], BF16, tag="tp", name="tp")
                    for mt, (m0, mc) in enumerate(CH):
                        nc.tensor.transpose(
                            psk[0:lc, m0 : m0 + mc], wkb[mt][:, l0 : l0 + lc], ident[0:mc, 0:mc]
                        )
                        nc.tensor.transpose(
                            psh[0:lc, m0 : m0 + mc], h1b[mt][:, l0 : l0 + lc], ident[0:mc, 0:mc]
                        )
                    kt_t = cpool.tile([lc, N1], BF16, tag=f"wkT{lt}", name=f"wkT{lt}")
                    ht_t = cpool.tile([lc, N1], BF16, tag=f"h1T{lt}", name=f"h1T{lt}")
                    nc.vector.tensor_copy(kt_t[:], psk[0:lc, 0:N1])
                    nc.vector.tensor_copy(ht_t[:], psh[0:lc, 0:N1])
                    wkT.append(kt_t)
                    h1T.append(ht_t)

                # W_hkT[m, j] = sum_l wk[m,l] h1[j,l]
                whkT = []
                for mt, (m0, mc) in enumerate(CH):
                    pw = psA.tile([128, 512], F32, tag="ps", name="ps_whk")
                    for lt, (l0, lc) in enumerate(CH):
                        nc.tensor.matmul(
                            pw[0:mc, 0:N1],
                            wkT[lt][:, m0 : m0 + mc],
                            h1T[lt][:],
                            start=(lt == 0),
                            stop=(lt == 2),
                        )
                    wTt = cpool.tile([mc, N1], BF16, tag=f"whkT{mt}", name=f"whkT{mt}")
                    nc.vector.tensor_copy(wTt[:], pw[0:mc, 0:N1])
                    whkT.append(wTt)

                # ---------------- x: load, de-interleave to bf16, transpose
                xnb = []
                for bt in range(4):
                    xt = xpool.tile([128, INP], F32, tag="xn", name="xn")
                    nc.sync.dma_start(xt[:], x[bt * 128 : (bt + 1) * 128, :])
                    xb = xbpool.tile([128, INP], BF16, tag="xnb", name="xnb")
                    nc.vector.tensor_copy(
                        xb[:].rearrange("p (i j) -> p i j", i=P1),
                        xt[:].rearrange("p (j i) -> p j i", i=P1).rearrange("p j i -> p i j"),
                    )
                    xnb.append(xb)

                xiT = [[None] * 3 for _ in range(P1)]
                for i in range(P1):
                    for c, (n0, cnt) in enumerate(CH):
                        ps = psT.tile([128, 512], BF16, tag="tp", name="tp")
                        for bt in range(4):
                            nc.tensor.transpose(
                                ps[0:cnt, bt * 128 : (bt + 1) * 128],
                                xnb[bt][:, i * N1 + n0 : i * N1 + n0 + cnt],
                                ident[:],
                            )
                        xi = xtpool.tile([cnt, BL], BF16, tag=f"xiT{i}_{c}", name=f"xiT{i}_{c}")
                        nc.vector.tensor_copy(xi[:], ps[0:cnt, :])
                        xiT[i][c] = xi

                # ---------------- logits + exp (densb accumulation) --------
                E = [[None] * 3 for _ in range(P1)]
                for jt, (j0, jc) in enumerate(CH):
                    for g0, gn in SG:
                        psts = []
                        for gi in range(gn):
                            pst = psA.tile([128, 512], F32, tag="ps", name="ps_st")
                            psts.append(pst)
                        for lt, (l0, lc) in enumerate(CH):
                            for gi in range(gn):
                                nc.tensor.matmul(
                                    psts[gi][0:jc, :],
                                    whkT[lt][:, j0 : j0 + jc],
                                    xiT[g0 + gi][lt][:],
                                    start=(lt == 0),
                                    stop=(lt == 2),
                                )
                        for gi in range(gn):
                            i = g0 + gi
                            ec = epool.tile([jc, BL], F32, tag=f"e{i}_{jt}", name=f"e{i}_{jt}")
                            col = i * 3 + jt
                            nc.scalar.activation(
                                ec[:],
                                psts[gi][0:jc, :],
                                AF.Exp,
                                bias=shiftc[0:jc, 0:1],
                                scale=SCALE,
                                accum_out=densb[0:jc, col : col + 1],
                            )
                            E[i][jt] = ec

                # ---------------- plw: my half -> deint bf16 -> rk[:, 0:512]
                # plwb col layout: 0:2016 de-interleaved (i*288+j), 2016 = plb
                # rk[i][c][p, s] = plw[s_glob, (n0+p)*7+i]; rkb row 0 = plb
                rk = [
                    [rkpool.tile([cnt, SEQ], BF16, tag=f"rk{i}_{c}", name=f"rk{i}_{c}") for c, (n0, cnt) in enumerate(CH)]
                    for i in range(P1)
                ]
                rkb = rkpool.tile([1, SEQ], BF16, tag="rkb", name="rkb")
                plws_writes = []
                for st in range(4):
                    pwf = plwfpool.tile([128, INP], F32, tag="plwf", name="plwf")
                    nc.sync.dma_start(
                        pwf[:], plw[bass.ds(par * 512 + st * 128, 128), :]
                    )
                    pwb = plwbpool.tile([128, 2017], BF16, tag="plwb", name="plwb")
                    eng = (nc.vector, nc.scalar, nc.vector, nc.scalar)[st]
                    if eng is nc.scalar:
                        nc.scalar.copy(
                            pwb[:, 0:INP].rearrange("p (i j) -> p i j", i=P1),
                            pwf[:].rearrange("p (j i) -> p j i", i=P1).rearrange("p j i -> p i j"),
                        )
                    else:
                        nc.vector.tensor_copy(
                            pwb[:, 0:INP].rearrange("p (i j) -> p i j", i=P1),
                            pwf[:].rearrange("p (j i) -> p j i", i=P1).rearrange("p j i -> p i j"),
                        )
                    # plb column
                    pbf = plwfpool.tile([128, 1], F32, tag="plbf", name="plbf")
                    nc.sync.dma_start(
                        pbf[:],
                        plb[bass.ds(par * 512 + st * 128, 128)].rearrange("(p a) -> p a", a=1),
                    )
                    nc.vector.tensor_copy(pwb[:, 2016:2017], pbf[:])

                    for i in range(P1):
                        for c, (n0, cnt) in enumerate(CH):
                            pst_ = psT.tile([128, 512], BF16, tag="tp", name="tp")
                            nc.tensor.transpose(
                                pst_[0:cnt, 0:128],
                                pwb[:, i * N1 + n0 : i * N1 + n0 + cnt],
                                ident[:],
                            )
                            nc.vector.tensor_copy(
                                rk[i][c][:, st * 128 : (st + 1) * 128], pst_[0:cnt, 0:128]
                            )
                    psb = psT.tile([128, 512], BF16, tag="tp", name="tp")
                    nc.tensor.transpose(psb[0:1, 0:128], pwb[:, 2016:2017], ident[:])
                    nc.vector.tensor_copy(rkb[0:1, st * 128 : (st + 1) * 128], psb[0:1, 0:128])
                    del pwf, pwb
                for i in range(P1):
                    for c in range(3):
                        w = nc.sync.dma_start(plws[i * 3 + c][par], rk[i][c][:, 0:512])
                        plws_writes.append(w)
                w = nc.sync.dma_start(plwsb[par], rkb[:, 0:512])
                plws_writes.append(w)

                # ---------------- AllReduce (also the plws sync barrier) ---
                ci = nc.gpsimd.dma_start(cc_in[:], densb[:])
                for w in plws_writes:
                    add_dep_helper(ci.ins, w.ins, reason="plws write before AR")
                cc = nc.gpsimd.collective_compute(
                    "AllReduce",
                    mybir.AluOpType.add,
                    replica_groups=[list(range(N_CORES))],
                    ins=[cc_in[:]],
                    outs=[cc_out[:]],
                )

                # ---------------- vT + ys~ (overlaps the AllReduce) --------
                for ntc, (n0, ncnt) in enumerate(CH):
                    for g0, gn in SG:
                        pvs = []
                        for gi in range(gn):
                            pv = psA.tile([128, 512], F32, tag="ps", name="ps_vt")
                            pvs.append(pv)
                        for mt, (m0, mc) in enumerate(CH):
                            for gi in range(gn):
                                nc.tensor.matmul(
                                    pvs[gi][0:ncnt, :],
                                    wvb[mt][:, n0 : n0 + ncnt],
                                    xiT[g0 + gi][mt][:],
                                    start=(mt == 0),
                                    stop=(mt == 2),
                                )
                        for gi in range(gn):
                            i = g0 + gi
                            nc.vector.tensor_mul(
                                E[i][ntc][:], pvs[gi][0:ncnt, :], E[i][ntc][:]
                            )

                # fetch buddy's plw^T half + the reduced denominators
                da = nc.gpsimd.dma_start(den_all[:], cc_out[:])
                add_dep_helper(da.ins, cc.ins, reason="den after AR")
                nc.vector.reciprocal(recip[:], den_all[:])
                for i in range(P1):
                    for c in range(3):
                        r = nc.sync.dma_start(rk[i][c][:, 512:1024], plws[i * 3 + c][other])
                        add_dep_helper(r.ins, cc.ins, reason="read buddy half after AR")
                r = nc.sync.dma_start(rkb[:, 512:1024], plwsb[other])
                add_dep_helper(r.ins, cc.ins, reason="read buddy half after AR")

            # ---------------- scan (normalize fused) + bf16 mirrors ---------
            with (
                tc.tile_pool(name="ysb", bufs=1) as ysbpool,
                tc.tile_pool(name="tmp", bufs=2) as tmppool,
                tc.tile_pool(name="osb", bufs=4) as outpool,
                tc.tile_pool(name="psP", bufs=8, space="PSUM") as psP,
            ):
                ysb = [
                    [ysbpool.tile([cnt, BL], BF16, tag=f"ysb{i}_{c}", name=f"ysb{i}_{c}") for c, (n0, cnt) in enumerate(CH)]
                    for i in range(P1)
                ]
                onesb = ysbpool.tile([1, BL], BF16, tag="onesb", name="onesb")
                nc.vector.memset(onesb[:], 1.0)

                for i in range(P1):
                    for c, (n0, cnt) in enumerate(CH):
                        col = i * 3 + c
                        nc.scalar.mul(
                            E[i][c][:], E[i][c][:], mul=recip[0:cnt, col : col + 1]
                        )
                    if i >= 1:
                        for c, (n0, cnt) in enumerate(CH):
                            tt = tmppool.tile([cnt, BL], F32, tag="tt", name="tt")
                            ts = tmppool.tile([cnt, BL], F32, tag="ts", name="ts")
                            nc.scalar.activation(
                                tt[:], E[i - 1][c][:], AF.Tanh,
                                bias=bcast[0:cnt, 2:3], scale=bcast[0:cnt, 0:1],
                            )
                            nc.scalar.activation(
                                ts[:], E[i - 1][c][:], AF.Sigmoid,
                                bias=bcast[0:cnt, 3:4], scale=bcast[0:cnt, 1:2],
                            )
                            nc.vector.tensor_mul(tt[:], tt[:], ts[:])
                            nc.vector.tensor_add(E[i][c][:], E[i][c][:], tt[:])
                    for c, (n0, cnt) in enumerate(CH):
                        if c == 1:
                            nc.scalar.copy(ysb[i][c][:], E[i][c][:])
                        else:
                            nc.vector.tensor_copy(ysb[i][c][:], E[i][c][:])

                # ---------------- projection: 21 ragged K-tiles + bias -----
                pps = [psP.tile([128, 512], F32, tag="pj", name="pj") for _ in range(8)]
                for bc in range(4):
                    for half in range(2):
                        nc.tensor.matmul(
                            pps[bc * 2 + half][:],
                            onesb[0:1, bc * 128 : (bc + 1) * 128],
                            rkb[0:1, half * 512 : (half + 1) * 512],
                            start=True,
                            stop=False,
                        )
                for i in range(P1):
                    for c, (n0, cnt) in enumerate(CH):
                        last = i == P1 - 1 and c == 2
                        for bc in range(4):
                            for half in range(2):
                                nc.tensor.matmul(
                                    pps[bc * 2 + half][:],
                                    ysb[i][c][:, bc * 128 : (bc + 1) * 128],
                                    rk[i][c][:, half * 512 : (half + 1) * 512],
                                    start=False,
                                    stop=last,
                                )
                for bc in range(4):
                    for half in range(2):
                        ob = outpool.tile([128, 512], F32, tag="osb", name="osb")
                        nc.vector.tensor_copy(ob[:], pps[bc * 2 + half][:])
                        off = (par if half == 0 else other) * 512
                        nc.sync.dma_start(
                            out[bc * 128 : (bc + 1) * 128, bass.ds(off, 512)], ob[:]
                        )

    nc.compile()
    return nc


_NC = None


def _get_nc():
    global _NC
    if _NC is None:
        _NC = build()
    return _NC


def run(inputs, trace=False):
    nc = _get_nc()
    rep_keys = [
        "w_k1",
        "w_v1",
        "h1",
        "alpha1",
        "alpha2",
        "beta1",
        "beta2",
        "proj_len_w",
        "proj_len_b",
    ]
    x = np.ascontiguousarray(inputs["x"], dtype=np.float32)
    rep = {k: np.ascontiguousarray(inputs[k], dtype=np.float32) for k in rep_keys}
    in_maps = [{"x": x[c * BL : (c + 1) * BL], **rep} for c in range(N_CORES)]
    res = run_bass_kernel_spmd(
        nc, in_maps, core_ids=list(range(N_CORES)), trace=trace
    )
    full = np.concatenate([res.results[c]["out"] for c in range(N_CORES)], axis=0)
    return full, res


def kernel(**inputs):
    full, _ = run(inputs, trace=False)
    return full
